# revision 7
# baseline (speedup 1.0000x reference)
"""Trainium2 Bass kernel for ChannelSqueezeSpatialAttention.

Reference computation (shapes hardcoded):
  xq  [4, 256, 64, 64], xkv [4, 256, 32, 32]
  wq/wk/wv [256, 256], emb_q/emb_k [17, 64, 3, 7, 7]
  q = wq @ xq (1x1 conv), k = wk @ xkv, v = wv @ xkv
  q_c = conv3d(q, emb_q) over (head, y, x) with kernel (3,7,7) -> 17 ch/head
  k_c = conv3d(k, emb_k)
  sim = softmax(q_c^T k_c / 8), rec = sim @ v  -> [4, 256, 64, 64]

Sharding: 8 cores = 4 batches x 2 head-pairs. Each core computes 2 heads of
one batch. The conv mixes adjacent heads (3-wide along head axis), so each
core computes q/k projections for its pair-relative head slots r0..r3 =
heads (2p-1, 2p, 2p+1, 2p+2); out-of-range slots get zero weight columns
host-side (no halo exchange needed).

Conv mapping: shift-and-accumulate matmuls with M = (ky, cg) = 7*17 = 119
packed output rows (padded to 128 for fast weight load). Both heads are
computed jointly per x-shift: h0 = wA.P01 + dz2.r2, h1 = wB.P23 + dz0.r1,
where the two 64-row extra taps sit in disjoint PE row groups (0:64 / 64:128)
and can co-execute. The ky-summation is deferred: partial planes are staged
to SBUF with a per-ky y-shift (SBUF->SBUF DMAs batched over 32-row supers),
and the scores matmul contracts over (ky, cg) with a ky-replicated k_c as
stationary, which completes the convolution for free.

Attention: scores computed transposed S^T[sk, sq] so softmax-exp output E^T
feeds the value matmul directly: rec^T[d|Z, sq] = [v|1]^T E^T. Division by
Z: DMA-reshape the Z row across 32 partitions, wide DVE reciprocal, DMA
back, K=1 broadcast matmul, DVE multiply.

Dtypes: the full matmul chain is bf16; PSUM accumulation stays fp32.
"""

import functools
import numpy as np
import ml_dtypes

import concourse.bass as bass
import concourse.tile as tile
import concourse.mybir as mybir
from concourse import bacc
from concourse.bass_utils import run_bass_kernel_spmd

F32 = mybir.dt.float32
BF16 = mybir.dt.bfloat16

B = 4
NH = 4
D = 64            # head dim
CG = 17           # squeezed channels
K7 = 7            # spatial kernel
HQ = 64           # q image h=w
HK = 32           # k image h=w
SQ = HQ * HQ      # 4096
SK = HK * HK      # 1024
MC = K7 * CG      # 119 conv output rows (ky, cg)
MCP = 128         # padded conv output rows
QP = HQ + 6       # 70: x-padded q row width
KP = HK + 6       # 38: x-padded k row width
SCALE = D ** -0.5

QCH = 8           # q spatial chunks (8 y-rows each)
QROWS = HQ // QCH  # 8
KCH = 2           # k spatial chunks (16 y-rows each)
KROWS = HK // KCH  # 16
NSLAB = SQ // 512  # 8 sq slabs per head

# blob layout (elements per partition, bf16)
OFF_WQ = 0          # [2, 256]
OFF_WK = 512        # [2, 256]
OFF_WV = 1024       # [2, 128]
OFF_CW = 1280       # 6 conv weight tensors [7, 128] each: qA qB qX kA kB kX
OFF_REPL = 6656     # [119] on first 119 partitions
OFF_XKV = 6784      # [2, 1024]
CBLOB = 8832

AF = mybir.ActivationFunctionType


def _build_program():
    nc = bacc.Bacc()

    blob = nc.dram_tensor("blob", [128, CBLOB], BF16, kind="ExternalInput")
    xq = nc.dram_tensor("xq", [256, SQ], BF16, kind="ExternalInput")
    out = nc.dram_tensor("out", [128, SQ], F32, kind="ExternalOutput")

    with tile.TileContext(nc) as tc:
        _emit(nc, tc, blob, xq, out)
    nc.compile()
    return nc


def _emit(nc, tc, blob, xq, out):
    import contextlib
    ctx = contextlib.ExitStack()
    with ctx:
        consts = ctx.enter_context(tc.tile_pool(name="consts", bufs=1))
        stg = ctx.enter_context(tc.tile_pool(name="stg", bufs=3))
        pqp = ctx.enter_context(tc.tile_pool(name="pqp", bufs=2))
        pkp = ctx.enter_context(tc.tile_pool(name="pkp", bufs=2))
        k2p = ctx.enter_context(tc.tile_pool(name="k2p", bufs=2))
        ep = ctx.enter_context(tc.tile_pool(name="ep", bufs=2))
        rp = ctx.enter_context(tc.tile_pool(name="rp", bufs=2))
        zp = ctx.enter_context(tc.tile_pool(name="zp", bufs=2))
        op = ctx.enter_context(tc.tile_pool(name="op", bufs=2))
        ps_mm = ctx.enter_context(tc.tile_pool(name="ps_mm", bufs=2, space="PSUM"))
        ps_sc = ctx.enter_context(tc.tile_pool(name="ps_sc", bufs=2, space="PSUM"))
        ps_rec = ctx.enter_context(tc.tile_pool(name="ps_rec", bufs=2, space="PSUM"))

        # ---- constant + input loads (2 big DMAs on separate queues) ----
        wb = consts.tile([128, CBLOB], BF16)
        nc.sync.dma_start(wb, blob[:])
        xq_sb = consts.tile([128, 2, SQ], BF16)
        nc.scalar.dma_start(xq_sb, xq.rearrange("(t p) n -> p t n", t=2))

        wqT_sb = wb[:, OFF_WQ:OFF_WQ + 512].rearrange("p (t m) -> p t m", t=2)
        wkT_sb = wb[:, OFF_WK:OFF_WK + 512].rearrange("p (t m) -> p t m", t=2)
        wvT_sb = wb[:, OFF_WV:OFF_WV + 256].rearrange("p (t m) -> p t m", t=2)
        cw = wb[:, OFF_CW:OFF_CW + 6 * 896].rearrange(
            "p (w x m) -> p w x m", w=6, x=7)
        wA = [cw[:, 0, :, :], cw[:, 3, :, :]]   # q, k: dz0@r-lo | dz1@r-hi
        wB = [cw[:, 1, :, :], cw[:, 4, :, :]]   # q, k: dz1@r-lo | dz2@r-hi
        wX = [cw[:, 2, :, :], cw[:, 5, :, :]]   # q, k: dz2(h0)@lo | dz0(h1)@hi
        repl_sb = wb[0:MC, OFF_REPL:OFF_REPL + MC]
        xkv_sb = wb[:, OFF_XKV:OFF_XKV + 2048].rearrange(
            "p (t n) -> p t n", t=2)

        # ones row at partition 0 (stationary for the 1/Z broadcast matmul)
        ones_t = consts.tile([1, 64], BF16)
        nc.vector.memset(ones_t[:], 1.0)

        # q/k plane tensors: A=[r0,r1], C=[r2,r3]; x-padded (64|32)x(70|38)
        qA = consts.tile([128, HQ * QP], BF16)
        qC = consts.tile([128, HQ * QP], BF16)
        kA = consts.tile([128, HK * KP], BF16)
        kC = consts.tile([128, HK * KP], BF16)
        for t in (qA, qC):
            v = t[:].rearrange("p (r c) -> p r c", c=QP)
            nc.vector.memset(v[:, :, 0:3], 0.0)
            nc.vector.memset(v[:, :, QP - 3:QP], 0.0)
        for t in (kA, kC):
            v = t[:].rearrange("p (r c) -> p r c", c=KP)
            nc.vector.memset(v[:, :, 0:3], 0.0)
            nc.vector.memset(v[:, :, KP - 3:KP], 0.0)

        # v^T projection: out[sk_blk, (h0 d |1|pad | h1 d |1|pad)], bf16,
        # padded to 128 cols per head so FWL triggers on the value matmul.
        v_sb = consts.tile([128, 8, 256], BF16)
        nc.vector.memset(v_sb[:, :, 64:128], 0.0)
        nc.vector.memset(v_sb[:, :, 192:256], 0.0)
        nc.vector.memset(v_sb[:, :, 64:65], 1.0)
        nc.vector.memset(v_sb[:, :, 192:193], 1.0)
        for t in range(8):
            acc = ps_mm.tile([128, 512], F32, tag="mm", name="accv")
            for ct in range(2):
                nc.tensor.matmul(
                    acc[:, 0:128],
                    xkv_sb[:, ct, t * 128:(t + 1) * 128],
                    wvT_sb[:, ct, :],
                    start=(ct == 0), stop=(ct == 1),
                )
            nc.vector.tensor_copy(v_sb[:, t, 0:64], acc[:, 0:64])
            nc.vector.tensor_copy(v_sb[:, t, 128:192], acc[:, 64:128])

        # k projection -> kA/kC padded planes (bf16)
        for ch in range(2):
            for mt in range(2):
                acc = ps_mm.tile([128, 512], F32, tag="mm", name="acckp")
                for ct in range(2):
                    nc.tensor.matmul(
                        acc[:],
                        wkT_sb[:, ct, mt * 128:(mt + 1) * 128],
                        xkv_sb[:, ct, ch * 512:(ch + 1) * 512],
                        start=(ct == 0), stop=(ct == 1),
                    )
                rows = 512 // HK  # 16
                y0 = ch * rows
                full = (kA, kC)[mt]
                dstf = full[:].rearrange("p (r c) -> p r c", c=KP)
                accv = acc[:].rearrange("p (r c) -> p r c", c=HK)
                nc.vector.tensor_copy(dstf[:, y0:y0 + rows, 3:3 + HK], accv)

        # q projection -> qA/qC (bf16)
        for ch in range(QCH):
            for mt in range(2):
                acc = ps_mm.tile([128, 512], F32, tag="mm", name="accqp")
                for ct in range(2):
                    nc.tensor.matmul(
                        acc[:],
                        wqT_sb[:, ct, mt * 128:(mt + 1) * 128],
                        xq_sb[:, ct, ch * 512:(ch + 1) * 512],
                        start=(ct == 0), stop=(ct == 1),
                    )
                rows = 512 // HQ  # 8
                y0 = ch * rows
                full = (qA, qC)[mt]
                dstf = full[:].rearrange("p (r c) -> p r c", c=QP)
                accv = acc[:].rearrange("p (r c) -> p r c", c=HQ)
                nc.vector.tensor_copy(dstf[:, y0:y0 + rows, 3:3 + HQ], accv)

        # ---- joint-head conv: both heads per x-shift, row-tiled extra taps
        pq_t = [None, None]
        pk_t = [None, None]
        k2_t = [None, None]
        st_t = [None, None]
        rec_sb_t = [None, None]

        def conv_pair(kind, ch):
            """One 512-col chunk of conv for BOTH heads (q: 8 rows, k: 16)."""
            if kind == 0:
                pA, pC, w_a, w_b, w_x = qA, qC, wA[0], wB[0], wX[0]
                rows, width, pw = QROWS, HQ, QP
            else:
                pA, pC, w_a, w_b, w_x = kA, kC, wA[1], wB[1], wX[1]
                rows, width, pw = KROWS, HK, KP
            y0 = ch * rows
            vA = pA[:].rearrange("p (r c) -> p r c", c=pw)
            vC = pC[:].rearrange("p (r c) -> p r c", c=pw)
            acc0 = ps_mm.tile([MCP, 512], F32, tag="mm", name="acc0")
            acc1 = ps_mm.tile([MCP, 512], F32, tag="mm", name="acc1")
            for kx in range(K7):
                winA = vA[:, y0:y0 + rows, kx:kx + width]
                winC = vC[:, y0:y0 + rows, kx:kx + width]
                nc.tensor.matmul(acc0[:], w_a[:, kx, :], winA,
                                 start=(kx == 0), stop=False)
                nc.tensor.matmul(acc1[:], w_b[:, kx, :], winC,
                                 start=(kx == 0), stop=False)
                # 64-row taps in disjoint PE row groups (co-execute)
                nc.tensor.matmul(acc0[:], w_x[0:64, kx, :],
                                 vC[0:64, y0:y0 + rows, kx:kx + width],
                                 start=False, stop=(kx == K7 - 1))
                nc.tensor.matmul(acc1[:], w_x[64:128, kx, :],
                                 vA[64:128, y0:y0 + rows, kx:kx + width],
                                 start=False, stop=(kx == K7 - 1))
            for h, acc in ((0, acc0), (1, acc1)):
                nc.vector.tensor_copy(
                    st_t[h][:, (ch % 4) * 512:(ch % 4 + 1) * 512],
                    acc[0:MC, :])

        def stage(kind, h, y0, rows):
            """Scatter one staged super (st rows y0..y0+rows) into pq/pk."""
            if kind == 0:
                dst, width, tot = pq_t[h], HQ, HQ
            else:
                dst, width, tot = pk_t[h], HK, HK
            st = st_t[h]
            for g in range(K7):
                i0 = max(0, g - 3 - y0)
                i1 = min(rows, tot + g - 3 - y0)
                if i1 <= i0:
                    continue
                cnt = (i1 - i0) * width
                d0 = (y0 + i0 - g + 3) * width
                nc.sync.dma_start(
                    dst[g * CG:(g + 1) * CG, d0:d0 + cnt],
                    st[g * CG:(g + 1) * CG, i0 * width:i0 * width + cnt],
                )

        def new_plane(kind, h):
            if kind == 0:
                pq = pqp.tile([MC, SQ], BF16, tag="pq%d" % h, name="pq")
                nc.vector.memset(pq[:, 0:3 * HQ], 0.0)
                nc.vector.memset(pq[:, (HQ - 3) * HQ:SQ], 0.0)
                pq_t[h] = pq
            else:
                pk = pkp.tile([MC, SK], BF16, tag="pk%d" % h, name="pk")
                nc.vector.memset(pk[:, 0:3 * HK], 0.0)
                nc.vector.memset(pk[:, (HK - 3) * HK:SK], 0.0)
                pk_t[h] = pk

        def k2_build(h):
            k2 = k2p.tile([MC, SK], BF16, tag="k2", name="k2")
            k2_t[h] = k2
            for ch in range(2):
                acc = ps_mm.tile([MC, 512], F32, tag="mm", name="acc2")
                nc.tensor.matmul(acc[:], repl_sb,
                                 pk_t[h][:, ch * 512:(ch + 1) * 512],
                                 start=True, stop=True)
                nc.vector.tensor_copy(k2[:, ch * 512:(ch + 1) * 512], acc[:])

        def slab(h, s):
            if s == 0:
                rec_sb_t[h] = rp.tile([65, SQ], F32, tag="recsb", name="recsb")
            e_sb = ep.tile([128, SQ], BF16, tag="e", name="esb")
            for quarter in range(4):
                sc = ps_sc.tile([128, 1024], F32, tag="sc", name="sc")
                for bb in range(2):
                    blk = quarter * 2 + bb
                    nc.tensor.matmul(
                        sc[:, bb * 512:(bb + 1) * 512],
                        k2_t[h][:, blk * 128:(blk + 1) * 128],
                        pq_t[h][:, s * 512:(s + 1) * 512],
                        start=True, stop=True,
                    )
                nc.scalar.activation(
                    e_sb[:, quarter * 1024:(quarter + 1) * 1024], sc[:],
                    AF.Exp, scale=SCALE)
            rec = ps_rec.tile([128, 512], F32, tag="rec", name="rec")
            for t in range(8):
                nc.tensor.matmul(
                    rec[:],
                    v_sb[:, t, h * 128:(h + 1) * 128],
                    e_sb[:, t * 512:(t + 1) * 512],
                    start=(t == 0), stop=(t == 7),
                )
            nc.vector.tensor_copy(
                rec_sb_t[h][:, s * 512:(s + 1) * 512], rec[0:65, :])

        def divide_q(h, qrt):
            rec_sb = rec_sb_t[h]
            c0 = qrt * 2048
            # 1/Z: spread the single-partition Z row across 32 partitions
            # via a small SBUF->SBUF DMA, wide DVE reciprocal, DMA back.
            zt = zp.tile([32, 64], F32, tag="zt", name="zt")
            nc.sync.dma_start(zt[:], rec_sb[64:65, c0:c0 + 2048])
            zi = zp.tile([32, 64], BF16, tag="zi", name="zi")
            with nc.allow_low_precision("1/Z in bf16: 0.4% rel err, fine"):
                nc.vector.reciprocal(zi[:], zt[:])
            zrow = zp.tile([1, 2048], BF16, tag="zrow", name="zrow")
            nc.sync.dma_start(zrow[:], zi[:])
            ot = op.tile([64, 2048], F32, tag="ot", name="ot")
            for j in range(4):
                s = qrt * 4 + j
                zb = ps_mm.tile([64, 512], F32, tag="mm", name="zb")
                nc.tensor.matmul(zb[:], ones_t[:],
                                 zrow[:, j * 512:(j + 1) * 512],
                                 start=True, stop=True)
                nc.vector.tensor_mul(
                    ot[:, j * 512:(j + 1) * 512],
                    rec_sb[0:64, s * 512:(s + 1) * 512], zb[:])
            nc.sync.dma_start(
                out[h * 64:(h + 1) * 64, c0:c0 + 2048], ot[:])

        # ---- schedule ----
        # k conv: one 32-row super per head's plane (2 chunks), then k2
        st_t[0] = stg.tile([MC, 2048], BF16, tag="stg0", name="st0")
        st_t[1] = stg.tile([MC, 2048], BF16, tag="stg1", name="st1")
        new_plane(1, 0)
        new_plane(1, 1)
        conv_pair(1, 0)
        conv_pair(1, 1)
        for h in range(2):
            stage(1, h, 0, HK)
        k2_build(0)
        k2_build(1)

        # q conv: 8 joint chunks; stage in 32-row supers after ch 3 and 7;
        # slabs interleave as their pq rows become available.
        new_plane(0, 0)
        new_plane(0, 1)
        st_t[0] = stg.tile([MC, 2048], BF16, tag="stg0", name="st0")
        st_t[1] = stg.tile([MC, 2048], BF16, tag="stg1", name="st1")
        for ch in range(4):
            conv_pair(0, ch)
        for h in range(2):
            stage(0, h, 0, 32)
        st_t[0] = stg.tile([MC, 2048], BF16, tag="stg0", name="st0")
        st_t[1] = stg.tile([MC, 2048], BF16, tag="stg1", name="st1")
        # slabs 0..2 of both heads are ready after super0; interleave with
        # the second super's conv chunks
        conv_pair(0, 4)
        slab(0, 0)
        conv_pair(0, 5)
        slab(1, 0)
        conv_pair(0, 6)
        slab(0, 1)
        conv_pair(0, 7)
        slab(1, 1)
        for h in range(2):
            stage(0, h, 32, 32)
        slab(0, 2)
        slab(1, 2)
        for s in range(3, NSLAB):
            slab(0, s)
            slab(1, s)
            if s == 3:
                divide_q(0, 0)
                divide_q(1, 0)
        divide_q(0, 1)
        divide_q(1, 1)


@functools.lru_cache(maxsize=1)
def _get_program():
    return _build_program()


def _host_inputs(xq, xkv, wq, wk, wv, emb_q, emb_k):
    """Build the 8 per-core input maps."""
    xq = np.ascontiguousarray(xq, dtype=np.float32)
    xkv = np.ascontiguousarray(xkv, dtype=np.float32)

    def conv_w(emb):
        # emb [cg, d, dnk, ky, kx] -> rows (dnk, d), cols (kx, ky, cg),
        # cols zero-padded 119 -> 128 for fast weight load.
        # Returns wA (dz0|dz1), wB (dz1|dz2), wX (dz2@lo | dz0@hi).
        arr = np.transpose(np.asarray(emb, np.float32), (2, 1, 4, 3, 0))
        arr = arr.reshape(3, 64, K7, MC)

        def pack(lo, hi):
            w = np.zeros((128, K7, MCP), np.float32)
            w[0:64, :, 0:MC] = lo
            w[64:128, :, 0:MC] = hi
            return w
        return (pack(arr[0], arr[1]), pack(arr[1], arr[2]),
                pack(arr[2], arr[0]))

    wAq, wBq, wXq = conv_w(emb_q)
    wAk, wBk, wXk = conv_w(emb_k)
    repl_ = np.tile(np.eye(CG, dtype=np.float32), (K7, K7))

    wq = np.asarray(wq, np.float32)
    wk = np.asarray(wk, np.float32)
    wv = np.asarray(wv, np.float32)

    in_maps = []
    for core in range(8):
        b, p = divmod(core, 2)
        wqT_ = np.zeros((256, 256), np.float32)
        wkT_ = np.zeros((256, 256), np.float32)
        for j in range(4):
            head = 2 * p + j - 1
            if 0 <= head < NH:
                wqT_[:, j * 64:(j + 1) * 64] = wq[head * 64:(head + 1) * 64, :].T
                wkT_[:, j * 64:(j + 1) * 64] = wk[head * 64:(head + 1) * 64, :].T
        wvT_ = wv[p * 128:(p + 1) * 128, :].T

        blob = np.zeros((128, CBLOB), np.float32)
        blob[:, OFF_WQ:OFF_WQ + 512] = \
            wqT_.reshape(2, 128, 256).transpose(1, 0, 2).reshape(128, 512)
        blob[:, OFF_WK:OFF_WK + 512] = \
            wkT_.reshape(2, 128, 256).transpose(1, 0, 2).reshape(128, 512)
        blob[:, OFF_WV:OFF_WV + 256] = \
            wvT_.reshape(2, 128, 128).transpose(1, 0, 2).reshape(128, 256)
        cws = np.stack([wAq, wBq, wXq, wAk, wBk, wXk], axis=1)  # [128,6,7,128]
        blob[:, OFF_CW:OFF_CW + 6 * 896] = cws.reshape(128, 6 * 896)
        blob[0:MC, OFF_REPL:OFF_REPL + MC] = repl_
        blob[:, OFF_XKV:OFF_XKV + 2048] = \
            xkv[b].reshape(256, SK).reshape(2, 128, SK).transpose(
                1, 0, 2).reshape(128, 2048)

        in_maps.append(dict(
            blob=np.ascontiguousarray(blob.astype(ml_dtypes.bfloat16)),
            xq=np.ascontiguousarray(
                xq[b].reshape(256, SQ).astype(ml_dtypes.bfloat16)),
        ))
    return in_maps


def _run(inputs, **kw):
    nc = _get_program()
    in_maps = _host_inputs(**inputs)
    res = run_bass_kernel_spmd(nc, in_maps, core_ids=list(range(8)), **kw)
    outp = np.empty((B, 256, HQ, HQ), np.float32)
    for core in range(8):
        b, p = divmod(core, 2)
        outp[b, p * 128:(p + 1) * 128] = \
            res.results[core]["out"].reshape(128, HQ, HQ)
    return outp, res


def kernel(xq, xkv, wq, wk, wv, emb_q, emb_k):
    outp, _ = _run(dict(xq=xq, xkv=xkv, wq=wq, wk=wk, wv=wv,
                        emb_q=emb_q, emb_k=emb_k))
    return outp


# revision 13
# speedup vs baseline: 1.1918x; 1.1918x over previous
"""Trainium2 Bass kernel for ChannelSqueezeSpatialAttention.

Reference computation (shapes hardcoded):
  xq  [4, 256, 64, 64], xkv [4, 256, 32, 32]
  wq/wk/wv [256, 256], emb_q/emb_k [17, 64, 3, 7, 7]
  q = wq @ xq (1x1 conv), k = wk @ xkv, v = wv @ xkv
  q_c = conv3d(q, emb_q) over (head, y, x) with kernel (3,7,7) -> 17 ch/head
  k_c = conv3d(k, emb_k)
  sim = softmax(q_c^T k_c / 8), rec = sim @ v  -> [4, 256, 64, 64]

Sharding: 8 cores = 4 batches x 2 head-pairs. Each core computes 2 heads of
one batch. The conv mixes adjacent heads (3-wide along head axis), so each
core computes q/k projections for its pair-relative head slots r0..r3 =
heads (2p-1, 2p, 2p+1, 2p+2); out-of-range slots get zero weight columns
host-side (no halo exchange needed).

Conv mapping: shift-and-accumulate matmuls with M = (ky, cg) = 7*17 = 119
packed output rows (padded to 128 for fast weight load). Both heads are
computed jointly per x-shift: h0 = wA.P01 + dz2.r2, h1 = wB.P23 + dz0.r1,
where the two 64-row extra taps sit in disjoint PE row groups (0:64 / 64:128)
and can co-execute. The ky-summation is deferred: partial planes are staged
to SBUF with a per-ky y-shift (SBUF->SBUF DMAs batched over 32-row supers),
and the scores matmul contracts over (ky, cg) with a ky-replicated k_c as
stationary, which completes the convolution for free.

Attention: scores computed transposed S^T[sk, sq] so softmax-exp output E^T
feeds the value matmul directly: rec^T[d|Z, sq] = [v|1]^T E^T. Division by
Z: DMA-reshape the Z row across 32 partitions, wide DVE reciprocal, DMA
back, K=1 broadcast matmul, DVE multiply.

Dtypes: the full matmul chain is bf16; PSUM accumulation stays fp32.
"""

import functools
import numpy as np
import ml_dtypes

import concourse.bass as bass
import concourse.tile as tile
import concourse.mybir as mybir
from concourse import bacc
from concourse.bass_utils import run_bass_kernel_spmd

F32 = mybir.dt.float32
BF16 = mybir.dt.bfloat16

B = 4
NH = 4
D = 64            # head dim
CG = 17           # squeezed channels
K7 = 7            # spatial kernel
HQ = 64           # q image h=w
HK = 32           # k image h=w
SQ = HQ * HQ      # 4096
SK = HK * HK      # 1024
MC = K7 * CG      # 119 conv output rows (ky, cg)
MCP = 128         # padded conv output rows
QP = HQ + 6       # 70: x-padded q row width
KP = HK + 6       # 38: x-padded k row width
SCALE = D ** -0.5

QCH = 8           # q spatial chunks (8 y-rows each)
QROWS = HQ // QCH  # 8
KCH = 2           # k spatial chunks (16 y-rows each)
KROWS = HK // KCH  # 16
NSLAB = SQ // 512  # 8 sq slabs per head

# blob layout (elements per partition, bf16); split into two DMAs so the
# projections can start before the conv weights arrive.
OFF_WQ = 0          # [2, 256]
OFF_WK = 512        # [2, 256]
OFF_WV = 1024       # [2, 128]
OFF_XKV = 1280      # [2, 1024]
BLOB1 = 3328
OFF_CW = 3328       # 6 conv weight tensors [7, 128] each: qA qB qX kA kB kX
OFF_REPL = 8704     # [119] on first 119 partitions
CBLOB = 8832

AF = mybir.ActivationFunctionType


def _build_program():
    nc = bacc.Bacc()

    blob = nc.dram_tensor("blob", [128, CBLOB], BF16, kind="ExternalInput")
    xq = nc.dram_tensor("xq", [256, SQ], BF16, kind="ExternalInput")
    out = nc.dram_tensor("out", [128, SQ], F32, kind="ExternalOutput")

    with tile.TileContext(nc) as tc:
        _emit(nc, tc, blob, xq, out)
    nc.compile()
    return nc


def _emit(nc, tc, blob, xq, out):
    import contextlib
    ctx = contextlib.ExitStack()
    with ctx:
        consts = ctx.enter_context(tc.tile_pool(name="consts", bufs=1))
        stg = ctx.enter_context(tc.tile_pool(name="stg", bufs=3))
        pqp = ctx.enter_context(tc.tile_pool(name="pqp", bufs=2))
        pkp = ctx.enter_context(tc.tile_pool(name="pkp", bufs=2))
        k2p = ctx.enter_context(tc.tile_pool(name="k2p", bufs=2))
        ep = ctx.enter_context(tc.tile_pool(name="ep", bufs=2))
        rp = ctx.enter_context(tc.tile_pool(name="rp", bufs=2))
        zp = ctx.enter_context(tc.tile_pool(name="zp", bufs=2))
        op = ctx.enter_context(tc.tile_pool(name="op", bufs=2))
        ps_mm = ctx.enter_context(tc.tile_pool(name="ps_mm", bufs=2, space="PSUM"))
        ps_sc = ctx.enter_context(tc.tile_pool(name="ps_sc", bufs=2, space="PSUM"))
        ps_rec = ctx.enter_context(tc.tile_pool(name="ps_rec", bufs=2, space="PSUM"))

        # ---- constant + input loads (3 big DMAs across both hwdge queues)
        wb = consts.tile([128, CBLOB], BF16)
        nc.sync.dma_start(wb[:, 0:BLOB1], blob[:, 0:BLOB1])
        nc.sync.dma_start(wb[:, BLOB1:CBLOB], blob[:, BLOB1:CBLOB])
        xq_sb = consts.tile([128, 2, SQ], BF16)
        nc.scalar.dma_start(xq_sb, xq.rearrange("(t p) n -> p t n", t=2))

        wqT_sb = wb[:, OFF_WQ:OFF_WQ + 512].rearrange("p (t m) -> p t m", t=2)
        wkT_sb = wb[:, OFF_WK:OFF_WK + 512].rearrange("p (t m) -> p t m", t=2)
        wvT_sb = wb[:, OFF_WV:OFF_WV + 256].rearrange("p (t m) -> p t m", t=2)
        cw = wb[:, OFF_CW:OFF_CW + 6 * 896].rearrange(
            "p (w x m) -> p w x m", w=6, x=7)
        wA = [cw[:, 0, :, :], cw[:, 3, :, :]]   # q, k: dz0@r-lo | dz1@r-hi
        wB = [cw[:, 1, :, :], cw[:, 4, :, :]]   # q, k: dz1@r-lo | dz2@r-hi
        wX = [cw[:, 2, :, :], cw[:, 5, :, :]]   # q, k: dz2(h0)@lo | dz0(h1)@hi
        repl_sb = wb[0:MC, OFF_REPL:OFF_REPL + MC]
        xkv_sb = wb[:, OFF_XKV:OFF_XKV + 2048].rearrange(
            "p (t n) -> p t n", t=2)

        # ones row at partition 0 (stationary for the 1/Z broadcast matmul)
        ones_t = consts.tile([1, 64], BF16)
        nc.vector.memset(ones_t[:], 1.0)

        # q/k plane tensors: A=[r0,r1], C=[r2,r3]; x-padded (64|32)x(70|38)
        qA = consts.tile([128, HQ * QP], BF16)
        qC = consts.tile([128, HQ * QP], BF16)
        kA = consts.tile([128, HK * KP], BF16)
        kC = consts.tile([128, HK * KP], BF16)
        for t in (qA, qC):
            v = t[:].rearrange("p (r c) -> p r c", c=QP)
            nc.vector.memset(v[:, :, 0:3], 0.0)
            nc.vector.memset(v[:, :, QP - 3:QP], 0.0)
        for t in (kA, kC):
            v = t[:].rearrange("p (r c) -> p r c", c=KP)
            nc.vector.memset(v[:, :, 0:3], 0.0)
            nc.vector.memset(v[:, :, KP - 3:KP], 0.0)

        # v^T projection: out[sk_blk, (h0 d |1|pad | h1 d |1|pad)], bf16,
        # padded to 128 cols per head so FWL triggers on the value matmul.
        v_sb = consts.tile([128, 8, 256], BF16)
        nc.vector.memset(v_sb[:, :, 64:128], 0.0)
        nc.vector.memset(v_sb[:, :, 192:256], 0.0)
        nc.vector.memset(v_sb[:, :, 64:65], 1.0)
        nc.vector.memset(v_sb[:, :, 192:193], 1.0)
        for t in range(8):
            acc = ps_mm.tile([128, 512], F32, tag="mm", name="accv")
            for ct in range(2):
                nc.tensor.matmul(
                    acc[:, 0:128],
                    xkv_sb[:, ct, t * 128:(t + 1) * 128],
                    wvT_sb[:, ct, :],
                    start=(ct == 0), stop=(ct == 1),
                )
            nc.vector.tensor_copy(v_sb[:, t, 0:64], acc[:, 0:64])
            nc.vector.tensor_copy(v_sb[:, t, 128:192], acc[:, 64:128])

        # k projection -> kA/kC padded planes (bf16)
        for ch in range(2):
            for mt in range(2):
                acc = ps_mm.tile([128, 512], F32, tag="mm", name="acckp")
                for ct in range(2):
                    nc.tensor.matmul(
                        acc[:],
                        wkT_sb[:, ct, mt * 128:(mt + 1) * 128],
                        xkv_sb[:, ct, ch * 512:(ch + 1) * 512],
                        start=(ct == 0), stop=(ct == 1),
                    )
                rows = 512 // HK  # 16
                y0 = ch * rows
                full = (kA, kC)[mt]
                dstf = full[:].rearrange("p (r c) -> p r c", c=KP)
                accv = acc[:].rearrange("p (r c) -> p r c", c=HK)
                nc.vector.tensor_copy(dstf[:, y0:y0 + rows, 3:3 + HK], accv)

        # q projection -> qA/qC (bf16)
        for ch in range(QCH):
            for mt in range(2):
                acc = ps_mm.tile([128, 512], F32, tag="mm", name="accqp")
                for ct in range(2):
                    nc.tensor.matmul(
                        acc[:],
                        wqT_sb[:, ct, mt * 128:(mt + 1) * 128],
                        xq_sb[:, ct, ch * 512:(ch + 1) * 512],
                        start=(ct == 0), stop=(ct == 1),
                    )
                rows = 512 // HQ  # 8
                y0 = ch * rows
                full = (qA, qC)[mt]
                dstf = full[:].rearrange("p (r c) -> p r c", c=QP)
                accv = acc[:].rearrange("p (r c) -> p r c", c=HQ)
                nc.vector.tensor_copy(dstf[:, y0:y0 + rows, 3:3 + HQ], accv)

        # ---- joint-head conv: both heads per x-shift, row-tiled extra taps
        pq_t = [None, None]
        pk_t = [None, None]
        k2_t = [None, None]
        st_t = [None, None]
        rec_sb_t = [None, None]

        def conv_pair(kind, ch):
            """One 512-col chunk of conv for BOTH heads (q: 8 rows, k: 16)."""
            if kind == 0:
                pA, pC, w_a, w_b, w_x = qA, qC, wA[0], wB[0], wX[0]
                rows, width, pw = QROWS, HQ, QP
            else:
                pA, pC, w_a, w_b, w_x = kA, kC, wA[1], wB[1], wX[1]
                rows, width, pw = KROWS, HK, KP
            y0 = ch * rows
            vA = pA[:].rearrange("p (r c) -> p r c", c=pw)
            vC = pC[:].rearrange("p (r c) -> p r c", c=pw)
            acc0 = ps_mm.tile([MCP, 512], F32, tag="mm", name="acc0")
            acc1 = ps_mm.tile([MCP, 512], F32, tag="mm", name="acc1")
            for kx in range(K7):
                winA = vA[:, y0:y0 + rows, kx:kx + width]
                winC = vC[:, y0:y0 + rows, kx:kx + width]
                nc.tensor.matmul(acc0[:], w_a[:, kx, :], winA,
                                 start=(kx == 0), stop=False)
                nc.tensor.matmul(acc1[:], w_b[:, kx, :], winC,
                                 start=(kx == 0), stop=False)
                # 64-row taps in disjoint PE row groups (co-execute)
                nc.tensor.matmul(acc0[:], w_x[0:64, kx, :],
                                 vC[0:64, y0:y0 + rows, kx:kx + width],
                                 start=False, stop=(kx == K7 - 1))
                nc.tensor.matmul(acc1[:], w_x[64:128, kx, :],
                                 vA[64:128, y0:y0 + rows, kx:kx + width],
                                 start=False, stop=(kx == K7 - 1))
            for h, acc in ((0, acc0), (1, acc1)):
                nc.vector.tensor_copy(
                    st_t[h][:, (ch % 2) * 512:(ch % 2 + 1) * 512],
                    acc[0:MC, :])

        def stage(kind, h, y0, rows):
            """Scatter one staged super (st rows y0..y0+rows) into pq/pk."""
            if kind == 0:
                dst, width, tot = pq_t[h], HQ, HQ
            else:
                dst, width, tot = pk_t[h], HK, HK
            st = st_t[h]
            for g in range(K7):
                i0 = max(0, g - 3 - y0)
                i1 = min(rows, tot + g - 3 - y0)
                if i1 <= i0:
                    continue
                cnt = (i1 - i0) * width
                d0 = (y0 + i0 - g + 3) * width
                nc.sync.dma_start(
                    dst[g * CG:(g + 1) * CG, d0:d0 + cnt],
                    st[g * CG:(g + 1) * CG, i0 * width:i0 * width + cnt],
                )

        def new_plane(kind, h):
            if kind == 0:
                pq = pqp.tile([MC, SQ], BF16, tag="pq%d" % h, name="pq")
                nc.vector.memset(pq[:, 0:3 * HQ], 0.0)
                nc.vector.memset(pq[:, (HQ - 3) * HQ:SQ], 0.0)
                pq_t[h] = pq
            else:
                pk = pkp.tile([MC, SK], BF16, tag="pk%d" % h, name="pk")
                nc.vector.memset(pk[:, 0:3 * HK], 0.0)
                nc.vector.memset(pk[:, (HK - 3) * HK:SK], 0.0)
                pk_t[h] = pk

        def k2_build(h):
            k2 = k2p.tile([MC, SK], BF16, tag="k2", name="k2")
            k2_t[h] = k2
            for ch in range(2):
                acc = ps_mm.tile([MC, 512], F32, tag="mm", name="acc2")
                nc.tensor.matmul(acc[:], repl_sb,
                                 pk_t[h][:, ch * 512:(ch + 1) * 512],
                                 start=True, stop=True)
                nc.vector.tensor_copy(k2[:, ch * 512:(ch + 1) * 512], acc[:])

        def slab(h, s):
            if s == 0:
                rec_sb_t[h] = rp.tile([65, SQ], F32, tag="recsb", name="recsb")
            e_sb = ep.tile([128, SQ], BF16, tag="e", name="esb")
            for quarter in range(4):
                sc = ps_sc.tile([128, 1024], F32, tag="sc", name="sc")
                for bb in range(2):
                    blk = quarter * 2 + bb
                    nc.tensor.matmul(
                        sc[:, bb * 512:(bb + 1) * 512],
                        k2_t[h][:, blk * 128:(blk + 1) * 128],
                        pq_t[h][:, s * 512:(s + 1) * 512],
                        start=True, stop=True,
                    )
                nc.scalar.activation(
                    e_sb[:, quarter * 1024:(quarter + 1) * 1024], sc[:],
                    AF.Exp, scale=SCALE)
            rec = ps_rec.tile([128, 512], F32, tag="rec", name="rec")
            for t in range(8):
                nc.tensor.matmul(
                    rec[:],
                    v_sb[:, t, h * 128:(h + 1) * 128],
                    e_sb[:, t * 512:(t + 1) * 512],
                    start=(t == 0), stop=(t == 7),
                )
            nc.vector.tensor_copy(
                rec_sb_t[h][:, s * 512:(s + 1) * 512], rec[0:65, :])

        zrow_t = {}

        def divide_prep(h, qrt):
            # 1/Z: spread the single-partition Z row across 32 partitions
            # via a small SBUF->SBUF DMA, wide DVE reciprocal, DMA back.
            # No tensor-engine work: safe to emit early.
            rec_sb = rec_sb_t[h]
            c0 = qrt * 2048
            zt = zp.tile([32, 64], F32, tag="zt", name="zt")
            nc.sync.dma_start(zt[:], rec_sb[64:65, c0:c0 + 2048])
            zi = zp.tile([32, 64], BF16, tag="zi", name="zi")
            with nc.allow_low_precision("1/Z in bf16: 0.4% rel err, fine"):
                nc.vector.reciprocal(zi[:], zt[:])
            zrow = zp.tile([1, 2048], BF16, tag="zrow", name="zrow")
            nc.sync.dma_start(zrow[:], zi[:])
            zrow_t[(h, qrt)] = zrow

        def divide_mm(h, qrt):
            rec_sb = rec_sb_t[h]
            c0 = qrt * 2048
            zrow = zrow_t[(h, qrt)]
            ot = op.tile([64, 2048], F32, tag="ot", name="ot")
            for j in range(4):
                s = qrt * 4 + j
                zb = ps_mm.tile([64, 512], F32, tag="mm", name="zb")
                nc.tensor.matmul(zb[:], ones_t[:],
                                 zrow[:, j * 512:(j + 1) * 512],
                                 start=True, stop=True)
                nc.vector.tensor_mul(
                    ot[:, j * 512:(j + 1) * 512],
                    rec_sb[0:64, s * 512:(s + 1) * 512], zb[:])
            nc.sync.dma_start(
                out[h * 64:(h + 1) * 64, c0:c0 + 2048], ot[:])

        # ---- schedule ----
        def new_st():
            st_t[0] = stg.tile([MC, 1024], BF16, tag="stg0", name="st0")
            st_t[1] = stg.tile([MC, 1024], BF16, tag="stg1", name="st1")

        # k conv: 2 joint chunks (16 rows each), staged as one 32-row super
        new_st()
        new_plane(1, 0)
        new_plane(1, 1)
        conv_pair(1, 0)
        conv_pair(1, 1)
        for h in range(2):
            stage(1, h, 0, HK)
        k2_build(0)
        k2_build(1)

        # q conv: 8 joint chunks, staged in 16-row supers (after ch 1,3,5,7).
        # Slabs are emitted lagging >=1 super behind their staging DMAs so
        # the tensor queue never stalls on staging; divide zb matmuls are
        # emitted >=2 slabs after their Z-prep chain.
        new_plane(0, 0)
        new_plane(0, 1)
        new_st()
        conv_pair(0, 0)
        conv_pair(0, 1)
        for h in range(2):
            stage(0, h, 0, 16)
        new_st()
        conv_pair(0, 2)
        conv_pair(0, 3)
        for h in range(2):
            stage(0, h, 16, 16)
        slab(0, 0)
        new_st()
        conv_pair(0, 4)
        slab(1, 0)
        conv_pair(0, 5)
        for h in range(2):
            stage(0, h, 32, 16)
        slab(0, 1)
        new_st()
        conv_pair(0, 6)
        slab(1, 1)
        conv_pair(0, 7)
        for h in range(2):
            stage(0, h, 48, 16)
        slab(0, 2)
        slab(1, 2)
        slab(0, 3)
        slab(1, 3)
        divide_prep(0, 0)
        divide_prep(1, 0)
        slab(0, 4)
        slab(1, 4)
        divide_mm(0, 0)
        slab(0, 5)
        divide_mm(1, 0)
        slab(1, 5)
        slab(0, 6)
        slab(1, 6)
        slab(0, 7)
        divide_prep(0, 1)
        slab(1, 7)
        divide_mm(0, 1)
        divide_prep(1, 1)
        divide_mm(1, 1)


@functools.lru_cache(maxsize=1)
def _get_program():
    return _build_program()


def _host_inputs(xq, xkv, wq, wk, wv, emb_q, emb_k):
    """Build the 8 per-core input maps."""
    xq = np.ascontiguousarray(xq, dtype=np.float32)
    xkv = np.ascontiguousarray(xkv, dtype=np.float32)

    def conv_w(emb):
        # emb [cg, d, dnk, ky, kx] -> rows (dnk, d), cols (kx, ky, cg),
        # cols zero-padded 119 -> 128 for fast weight load.
        # Returns wA (dz0|dz1), wB (dz1|dz2), wX (dz2@lo | dz0@hi).
        arr = np.transpose(np.asarray(emb, np.float32), (2, 1, 4, 3, 0))
        arr = arr.reshape(3, 64, K7, MC)

        def pack(lo, hi):
            w = np.zeros((128, K7, MCP), np.float32)
            w[0:64, :, 0:MC] = lo
            w[64:128, :, 0:MC] = hi
            return w
        return (pack(arr[0], arr[1]), pack(arr[1], arr[2]),
                pack(arr[2], arr[0]))

    wAq, wBq, wXq = conv_w(emb_q)
    wAk, wBk, wXk = conv_w(emb_k)
    repl_ = np.tile(np.eye(CG, dtype=np.float32), (K7, K7))

    wq = np.asarray(wq, np.float32)
    wk = np.asarray(wk, np.float32)
    wv = np.asarray(wv, np.float32)

    in_maps = []
    for core in range(8):
        b, p = divmod(core, 2)
        wqT_ = np.zeros((256, 256), np.float32)
        wkT_ = np.zeros((256, 256), np.float32)
        for j in range(4):
            head = 2 * p + j - 1
            if 0 <= head < NH:
                wqT_[:, j * 64:(j + 1) * 64] = wq[head * 64:(head + 1) * 64, :].T
                wkT_[:, j * 64:(j + 1) * 64] = wk[head * 64:(head + 1) * 64, :].T
        wvT_ = wv[p * 128:(p + 1) * 128, :].T

        blob = np.zeros((128, CBLOB), np.float32)
        blob[:, OFF_WQ:OFF_WQ + 512] = \
            wqT_.reshape(2, 128, 256).transpose(1, 0, 2).reshape(128, 512)
        blob[:, OFF_WK:OFF_WK + 512] = \
            wkT_.reshape(2, 128, 256).transpose(1, 0, 2).reshape(128, 512)
        blob[:, OFF_WV:OFF_WV + 256] = \
            wvT_.reshape(2, 128, 128).transpose(1, 0, 2).reshape(128, 256)
        blob[:, OFF_XKV:OFF_XKV + 2048] = \
            xkv[b].reshape(2, 128, SK).transpose(1, 0, 2).reshape(128, 2048)
        cws = np.stack([wAq, wBq, wXq, wAk, wBk, wXk], axis=1)  # [128,6,7,128]
        blob[:, OFF_CW:OFF_CW + 6 * 896] = cws.reshape(128, 6 * 896)
        blob[0:MC, OFF_REPL:OFF_REPL + MC] = repl_

        in_maps.append(dict(
            blob=np.ascontiguousarray(blob.astype(ml_dtypes.bfloat16)),
            xq=np.ascontiguousarray(
                xq[b].reshape(256, SQ).astype(ml_dtypes.bfloat16)),
        ))
    return in_maps


def _run(inputs, **kw):
    nc = _get_program()
    in_maps = _host_inputs(**inputs)
    res = run_bass_kernel_spmd(nc, in_maps, core_ids=list(range(8)), **kw)
    outp = np.empty((B, 256, HQ, HQ), np.float32)
    for core in range(8):
        b, p = divmod(core, 2)
        outp[b, p * 128:(p + 1) * 128] = \
            res.results[core]["out"].reshape(128, HQ, HQ)
    return outp, res


def kernel(xq, xkv, wq, wk, wv, emb_q, emb_k):
    outp, _ = _run(dict(xq=xq, xkv=xkv, wq=wq, wk=wk, wv=wv,
                        emb_q=emb_q, emb_k=emb_k))
    return outp


# revision 18
# speedup vs baseline: 1.2850x; 1.0782x over previous
"""Trainium2 Bass kernel for ChannelSqueezeSpatialAttention.

Reference computation (shapes hardcoded):
  xq  [4, 256, 64, 64], xkv [4, 256, 32, 32]
  wq/wk/wv [256, 256], emb_q/emb_k [17, 64, 3, 7, 7]
  q = wq @ xq (1x1 conv), k = wk @ xkv, v = wv @ xkv
  q_c = conv3d(q, emb_q) over (head, y, x) with kernel (3,7,7) -> 17 ch/head
  k_c = conv3d(k, emb_k)
  sim = softmax(q_c^T k_c / 8), rec = sim @ v  -> [4, 256, 64, 64]

Sharding: 8 cores = 4 batches x 2 head-pairs. Each core computes 2 heads of
one batch. The conv mixes adjacent heads (3-wide along head axis), so each
core computes q/k projections for its pair-relative head slots r0..r3 =
heads (2p-1, 2p, 2p+1, 2p+2); out-of-range slots get zero weight columns
host-side (no halo exchange needed).

Conv mapping: shift-and-accumulate matmuls with M = (ky, cg) = 7*17 = 119
packed output rows (padded to 128 for fast weight load). Both heads are
computed jointly per x-shift: h0 = wA.P01 + dz2.r2, h1 = wB.P23 + dz0.r1,
where the two 64-row extra taps sit in disjoint PE row groups (0:64 / 64:128)
and can co-execute. The ky-summation is deferred: partial planes are staged
to SBUF with a per-ky y-shift (SBUF->SBUF DMAs batched over 32-row supers),
and the scores matmul contracts over (ky, cg) with a ky-replicated k_c as
stationary, which completes the convolution for free.

Attention: scores computed transposed S^T[sk, sq] so softmax-exp output E^T
feeds the value matmul directly: rec^T[d|Z, sq] = [v|1]^T E^T. Division by
Z: DMA-reshape the Z row across 32 partitions, wide DVE reciprocal, DMA
back, K=1 broadcast matmul, DVE multiply.

Dtypes: the full matmul chain is bf16; PSUM accumulation stays fp32.
"""

import functools
import numpy as np
import ml_dtypes

import concourse.bass as bass
import concourse.tile as tile
import concourse.mybir as mybir
from concourse import bacc
from concourse.bass_utils import run_bass_kernel_spmd

F32 = mybir.dt.float32
BF16 = mybir.dt.bfloat16

B = 4
NH = 4
D = 64            # head dim
CG = 17           # squeezed channels
K7 = 7            # spatial kernel
HQ = 64           # q image h=w
HK = 32           # k image h=w
SQ = HQ * HQ      # 4096
SK = HK * HK      # 1024
MC = K7 * CG      # 119 conv output rows (ky, cg)
MCP = 128         # padded conv output rows
QP = HQ + 6       # 70: x-padded q row width
KP = HK + 6       # 38: x-padded k row width
SCALE = D ** -0.5

QCH = 8           # q spatial chunks (8 y-rows each)
QROWS = HQ // QCH  # 8
KCH = 2           # k spatial chunks (16 y-rows each)
KROWS = HK // KCH  # 16
NSLAB = SQ // 512  # 8 sq slabs per head

# blob layout (elements per partition, bf16); split into two DMAs so the
# projections can start before the conv weights arrive.
OFF_WQ = 0          # [2, 256]
OFF_WK = 512        # [2, 256]
OFF_WV = 1024       # [2, 128]
OFF_XKV = 1280      # [2, 1024]
BLOB1 = 3328
OFF_CW = 3328       # 6 conv weight tensors [7, 128] each: qA qB qX kA kB kX
OFF_REPL = 8704     # [119] on first 119 partitions
CBLOB = 8832

AF = mybir.ActivationFunctionType


def _build_program():
    nc = bacc.Bacc()

    blob = nc.dram_tensor("blob", [128, CBLOB], BF16, kind="ExternalInput")
    xq = nc.dram_tensor("xq", [256, SQ], BF16, kind="ExternalInput")
    out = nc.dram_tensor("out", [128, SQ], F32, kind="ExternalOutput")

    with tile.TileContext(nc) as tc:
        _emit(nc, tc, blob, xq, out)
    nc.compile()
    return nc


def _emit(nc, tc, blob, xq, out):
    import contextlib
    ctx = contextlib.ExitStack()
    with ctx:
        consts = ctx.enter_context(tc.tile_pool(name="consts", bufs=1))
        stg = ctx.enter_context(tc.tile_pool(name="stg", bufs=3))
        pqp = ctx.enter_context(tc.tile_pool(name="pqp", bufs=2))
        pkp = ctx.enter_context(tc.tile_pool(name="pkp", bufs=2))
        k2p = ctx.enter_context(tc.tile_pool(name="k2p", bufs=2))
        ep = ctx.enter_context(tc.tile_pool(name="ep", bufs=2))
        rp = ctx.enter_context(tc.tile_pool(name="rp", bufs=2))
        zp = ctx.enter_context(tc.tile_pool(name="zp", bufs=2))
        op = ctx.enter_context(tc.tile_pool(name="op", bufs=2))
        ps_mm = ctx.enter_context(tc.tile_pool(name="ps_mm", bufs=2, space="PSUM"))
        ps_sc = ctx.enter_context(tc.tile_pool(name="ps_sc", bufs=2, space="PSUM"))
        ps_rec = ctx.enter_context(tc.tile_pool(name="ps_rec", bufs=2, space="PSUM"))

        # ---- constant + input loads across both hwdge queues; k/v weights
        # and xkv first so the k/v projections start earliest.
        wb = consts.tile([128, CBLOB], BF16)
        nc.sync.dma_start(wb[:, OFF_WK:BLOB1], blob[:, OFF_WK:BLOB1])
        nc.sync.dma_start(wb[:, OFF_WQ:OFF_WQ + 512],
                          blob[:, OFF_WQ:OFF_WQ + 512])
        nc.sync.dma_start(wb[:, BLOB1:CBLOB], blob[:, BLOB1:CBLOB])
        xq_sb = consts.tile([128, 2, SQ], BF16)
        nc.scalar.dma_start(xq_sb, xq.rearrange("(t p) n -> p t n", t=2))

        wqT_sb = wb[:, OFF_WQ:OFF_WQ + 512].rearrange("p (t m) -> p t m", t=2)
        wkT_sb = wb[:, OFF_WK:OFF_WK + 512].rearrange("p (t m) -> p t m", t=2)
        wvT_sb = wb[:, OFF_WV:OFF_WV + 256].rearrange("p (t m) -> p t m", t=2)
        cw = wb[:, OFF_CW:OFF_CW + 6 * 896].rearrange(
            "p (w x m) -> p w x m", w=6, x=7)
        wA = [cw[:, 0, :, :], cw[:, 3, :, :]]   # q, k: dz0@r-lo | dz1@r-hi
        wB = [cw[:, 1, :, :], cw[:, 4, :, :]]   # q, k: dz1@r-lo | dz2@r-hi
        wX = [cw[:, 2, :, :], cw[:, 5, :, :]]   # q, k: dz2(h0)@lo | dz0(h1)@hi
        repl_sb = wb[0:MC, OFF_REPL:OFF_REPL + MC]
        xkv_sb = wb[:, OFF_XKV:OFF_XKV + 2048].rearrange(
            "p (t n) -> p t n", t=2)

        # ones row at partition 0 (stationary for the 1/Z broadcast matmul)
        ones_t = consts.tile([1, 64], BF16)
        nc.vector.memset(ones_t[:], 1.0)

        # q/k plane tensors: A=[r0,r1], C=[r2,r3], X=[r2,r1] (the X plane
        # lets both heads' 64-row extra taps read ONE tensor at the same
        # column address, so the row-tiled pair truly co-streams).
        qA = consts.tile([128, HQ * QP], BF16)
        qC = consts.tile([128, HQ * QP], BF16)
        qX = consts.tile([128, HQ * QP], BF16)
        kA = consts.tile([128, HK * KP], BF16)
        kC = consts.tile([128, HK * KP], BF16)
        kX = consts.tile([128, HK * KP], BF16)
        for t in (qA, qC):
            v = t[:].rearrange("p (r c) -> p r c", c=QP)
            nc.vector.memset(v[:, :, 0:3], 0.0)
            nc.vector.memset(v[:, :, QP - 3:QP], 0.0)
        for t in (kA, kC):
            v = t[:].rearrange("p (r c) -> p r c", c=KP)
            nc.vector.memset(v[:, :, 0:3], 0.0)
            nc.vector.memset(v[:, :, KP - 3:KP], 0.0)

        # v^T projection: out[sk_blk, (h0 d |1|pad | h1 d |1|pad)], bf16,
        # padded to 128 cols per head so FWL triggers on the value matmul.
        v_sb = consts.tile([128, 8, 256], BF16)
        nc.vector.memset(v_sb[:, :, 64:128], 0.0)
        nc.vector.memset(v_sb[:, :, 192:256], 0.0)
        nc.vector.memset(v_sb[:, :, 64:65], 1.0)
        nc.vector.memset(v_sb[:, :, 192:193], 1.0)
        for t in range(8):
            acc = ps_mm.tile([128, 512], F32, tag="mm", name="accv")
            for ct in range(2):
                nc.tensor.matmul(
                    acc[:, 0:128],
                    xkv_sb[:, ct, t * 128:(t + 1) * 128],
                    wvT_sb[:, ct, :],
                    start=(ct == 0), stop=(ct == 1),
                )
            nc.vector.tensor_copy(v_sb[:, t, 0:64], acc[:, 0:64])
            nc.vector.tensor_copy(v_sb[:, t, 128:192], acc[:, 64:128])

        # k projection -> kA/kC padded planes (bf16)
        for ch in range(2):
            for mt in range(2):
                acc = ps_mm.tile([128, 512], F32, tag="mm", name="acckp")
                for ct in range(2):
                    nc.tensor.matmul(
                        acc[:],
                        wkT_sb[:, ct, mt * 128:(mt + 1) * 128],
                        xkv_sb[:, ct, ch * 512:(ch + 1) * 512],
                        start=(ct == 0), stop=(ct == 1),
                    )
                rows = 512 // HK  # 16
                y0 = ch * rows
                full = (kA, kC)[mt]
                dstf = full[:].rearrange("p (r c) -> p r c", c=KP)
                accv = acc[:].rearrange("p (r c) -> p r c", c=HK)
                nc.vector.tensor_copy(dstf[:, y0:y0 + rows, 3:3 + HK], accv)

        # q projection -> qA/qC (bf16)
        for ch in range(QCH):
            for mt in range(2):
                acc = ps_mm.tile([128, 512], F32, tag="mm", name="accqp")
                for ct in range(2):
                    nc.tensor.matmul(
                        acc[:],
                        wqT_sb[:, ct, mt * 128:(mt + 1) * 128],
                        xq_sb[:, ct, ch * 512:(ch + 1) * 512],
                        start=(ct == 0), stop=(ct == 1),
                    )
                rows = 512 // HQ  # 8
                y0 = ch * rows
                full = (qA, qC)[mt]
                dstf = full[:].rearrange("p (r c) -> p r c", c=QP)
                accv = acc[:].rearrange("p (r c) -> p r c", c=HQ)
                nc.vector.tensor_copy(dstf[:, y0:y0 + rows, 3:3 + HQ], accv)

        # ---- joint-head conv: both heads per x-shift, row-tiled extra taps
        pq_t = [None, None]
        pk_t = [None, None]
        k2_t = [None, None]
        st_t = [None, None]
        rec_sb_t = [None, None]

        def conv_pair(kind, ch, st):
            """One 512-col chunk of conv for BOTH heads (q: 8 rows, k: 16)."""
            if kind == 0:
                pA, pC, pX, w_a, w_b, w_x = qA, qC, qX, wA[0], wB[0], wX[0]
                rows, width, pw = QROWS, HQ, QP
            else:
                pA, pC, pX, w_a, w_b, w_x = kA, kC, kX, wA[1], wB[1], wX[1]
                rows, width, pw = KROWS, HK, KP
            y0 = ch * rows
            vA = pA[:].rearrange("p (r c) -> p r c", c=pw)
            vC = pC[:].rearrange("p (r c) -> p r c", c=pw)
            vX = pX[:].rearrange("p (r c) -> p r c", c=pw)
            acc0 = ps_mm.tile([MCP, 512], F32, tag="mm", name="acc0")
            acc1 = ps_mm.tile([MCP, 512], F32, tag="mm", name="acc1")
            for kx in range(K7):
                winA = vA[:, y0:y0 + rows, kx:kx + width]
                winC = vC[:, y0:y0 + rows, kx:kx + width]
                nc.tensor.matmul(acc0[:], w_a[:, kx, :], winA,
                                 start=(kx == 0), stop=False)
                nc.tensor.matmul(acc1[:], w_b[:, kx, :], winC,
                                 start=(kx == 0), stop=False)
                # 64-row taps in disjoint PE row groups reading one plane
                # at the same column window: they co-stream.
                nc.tensor.matmul(acc0[:], w_x[0:64, kx, :],
                                 vX[0:64, y0:y0 + rows, kx:kx + width],
                                 start=False, stop=(kx == K7 - 1))
                nc.tensor.matmul(acc1[:], w_x[64:128, kx, :],
                                 vX[64:128, y0:y0 + rows, kx:kx + width],
                                 start=False, stop=(kx == K7 - 1))
            for h, acc in ((0, acc0), (1, acc1)):
                nc.vector.tensor_copy(
                    st[h][:, (ch % 2) * 512:(ch % 2 + 1) * 512],
                    acc[0:MC, :])

        def stage(kind, h, y0, rows, stp):
            """Scatter one staged super (st rows y0..y0+rows) into pq/pk."""
            if kind == 0:
                dst, width, tot = pq_t[h], HQ, HQ
            else:
                dst, width, tot = pk_t[h], HK, HK
            st = stp[h]
            for g in range(K7):
                i0 = max(0, g - 3 - y0)
                i1 = min(rows, tot + g - 3 - y0)
                if i1 <= i0:
                    continue
                cnt = (i1 - i0) * width
                d0 = (y0 + i0 - g + 3) * width
                nc.sync.dma_start(
                    dst[g * CG:(g + 1) * CG, d0:d0 + cnt],
                    st[g * CG:(g + 1) * CG, i0 * width:i0 * width + cnt],
                )

        def new_plane(kind, h):
            if kind == 0:
                pq = pqp.tile([MC, SQ], BF16, tag="pq%d" % h, name="pq")
                nc.vector.memset(pq[:, 0:3 * HQ], 0.0)
                nc.vector.memset(pq[:, (HQ - 3) * HQ:SQ], 0.0)
                pq_t[h] = pq
            else:
                pk = pkp.tile([MC, SK], BF16, tag="pk%d" % h, name="pk")
                nc.vector.memset(pk[:, 0:3 * HK], 0.0)
                nc.vector.memset(pk[:, (HK - 3) * HK:SK], 0.0)
                pk_t[h] = pk

        def k2_build(h):
            k2 = k2p.tile([MC, SK], BF16, tag="k2", name="k2")
            k2_t[h] = k2
            for ch in range(2):
                acc = ps_mm.tile([MC, 512], F32, tag="mm", name="acc2")
                nc.tensor.matmul(acc[:], repl_sb,
                                 pk_t[h][:, ch * 512:(ch + 1) * 512],
                                 start=True, stop=True)
                nc.vector.tensor_copy(k2[:, ch * 512:(ch + 1) * 512], acc[:])

        def slab(h, s):
            if s == 0:
                rec_sb_t[h] = rp.tile([65, SQ], F32, tag="recsb", name="recsb")
            e_sb = ep.tile([128, SQ], BF16, tag="e", name="esb")
            for quarter in range(4):
                sc = ps_sc.tile([128, 1024], F32, tag="sc", name="sc")
                for bb in range(2):
                    blk = quarter * 2 + bb
                    nc.tensor.matmul(
                        sc[:, bb * 512:(bb + 1) * 512],
                        k2_t[h][:, blk * 128:(blk + 1) * 128],
                        pq_t[h][:, s * 512:(s + 1) * 512],
                        start=True, stop=True,
                    )
                nc.scalar.activation(
                    e_sb[:, quarter * 1024:(quarter + 1) * 1024], sc[:],
                    AF.Exp, scale=SCALE)
            rec = ps_rec.tile([128, 512], F32, tag="rec", name="rec")
            for t in range(8):
                nc.tensor.matmul(
                    rec[:],
                    v_sb[:, t, h * 128:(h + 1) * 128],
                    e_sb[:, t * 512:(t + 1) * 512],
                    start=(t == 0), stop=(t == 7),
                )
            nc.vector.tensor_copy(
                rec_sb_t[h][:, s * 512:(s + 1) * 512], rec[0:65, :])

        zrow_t = {}

        def divide_prep(h, qrt):
            # 1/Z: spread the single-partition Z row across 32 partitions
            # via a small SBUF->SBUF DMA, wide DVE reciprocal, DMA back.
            # No tensor-engine work: safe to emit early.
            rec_sb = rec_sb_t[h]
            c0 = qrt * 2048
            zt = zp.tile([32, 64], F32, tag="zt", name="zt")
            nc.sync.dma_start(zt[:], rec_sb[64:65, c0:c0 + 2048])
            zi = zp.tile([32, 64], BF16, tag="zi", name="zi")
            with nc.allow_low_precision("1/Z in bf16: 0.4% rel err, fine"):
                nc.vector.reciprocal(zi[:], zt[:])
            zrow = zp.tile([1, 2048], BF16, tag="zrow", name="zrow")
            nc.sync.dma_start(zrow[:], zi[:])
            zrow_t[(h, qrt)] = zrow

        def divide_mm(h, qrt):
            rec_sb = rec_sb_t[h]
            c0 = qrt * 2048
            zrow = zrow_t[(h, qrt)]
            ot = op.tile([64, 2048], F32, tag="ot", name="ot")
            for j in range(4):
                s = qrt * 4 + j
                zb = ps_mm.tile([64, 512], F32, tag="mm", name="zb")
                nc.tensor.matmul(zb[:], ones_t[:],
                                 zrow[:, j * 512:(j + 1) * 512],
                                 start=True, stop=True)
                nc.vector.tensor_mul(
                    ot[:, j * 512:(j + 1) * 512],
                    rec_sb[0:64, s * 512:(s + 1) * 512], zb[:])
            nc.sync.dma_start(
                out[h * 64:(h + 1) * 64, c0:c0 + 2048], ot[:])

        # ---- schedule ----
        def new_st():
            return (stg.tile([MC, 1024], BF16, tag="stg0", name="st0"),
                    stg.tile([MC, 1024], BF16, tag="stg1", name="st1"))

        # X planes = [r2 | r1] built by partition-moving SBUF DMAs
        nc.sync.dma_start(kX[0:64, :], kC[0:64, :])
        nc.sync.dma_start(kX[64:128, :], kA[64:128, :])
        nc.sync.dma_start(qX[0:64, :], qC[0:64, :])
        nc.sync.dma_start(qX[64:128, :], qA[64:128, :])

        # k conv (2 joint 16-row chunks); its staging and k2 are emitted
        # between early q-conv chunks so the tensor queue never waits on
        # staging DMA dispatch. q staged in 16-row supers; slabs lag >=1
        # super; divide zb matmuls lag >=2 slabs behind their Z-prep.
        stk = new_st()
        new_plane(1, 0)
        new_plane(1, 1)
        conv_pair(1, 0, stk)
        conv_pair(1, 1, stk)
        new_plane(0, 0)
        new_plane(0, 1)
        st0 = new_st()
        conv_pair(0, 0, st0)
        for h in range(2):
            stage(1, h, 0, HK, stk)
        conv_pair(0, 1, st0)
        for h in range(2):
            stage(0, h, 0, 16, st0)
        k2_build(0)
        k2_build(1)
        st1 = new_st()
        conv_pair(0, 2, st1)
        conv_pair(0, 3, st1)
        for h in range(2):
            stage(0, h, 16, 16, st1)
        slab(0, 0)
        st2 = new_st()
        conv_pair(0, 4, st2)
        slab(1, 0)
        conv_pair(0, 5, st2)
        for h in range(2):
            stage(0, h, 32, 16, st2)
        slab(0, 1)
        st3 = new_st()
        conv_pair(0, 6, st3)
        slab(1, 1)
        conv_pair(0, 7, st3)
        for h in range(2):
            stage(0, h, 48, 16, st3)
        slab(0, 2)
        slab(1, 2)
        slab(0, 3)
        slab(1, 3)
        divide_prep(0, 0)
        divide_prep(1, 0)
        slab(0, 4)
        slab(1, 4)
        divide_mm(0, 0)
        slab(0, 5)
        divide_mm(1, 0)
        slab(1, 5)
        slab(0, 6)
        slab(0, 7)
        divide_prep(0, 1)
        slab(1, 6)
        slab(1, 7)
        divide_mm(0, 1)
        divide_prep(1, 1)
        divide_mm(1, 1)


@functools.lru_cache(maxsize=1)
def _get_program():
    return _build_program()


def _host_inputs(xq, xkv, wq, wk, wv, emb_q, emb_k):
    """Build the 8 per-core input maps."""
    xq = np.ascontiguousarray(xq, dtype=np.float32)
    xkv = np.ascontiguousarray(xkv, dtype=np.float32)

    def conv_w(emb):
        # emb [cg, d, dnk, ky, kx] -> rows (dnk, d), cols (kx, ky, cg),
        # cols zero-padded 119 -> 128 for fast weight load.
        # Returns wA (dz0|dz1), wB (dz1|dz2), wX (dz2@lo | dz0@hi).
        arr = np.transpose(np.asarray(emb, np.float32), (2, 1, 4, 3, 0))
        arr = arr.reshape(3, 64, K7, MC)

        def pack(lo, hi):
            w = np.zeros((128, K7, MCP), np.float32)
            w[0:64, :, 0:MC] = lo
            w[64:128, :, 0:MC] = hi
            return w
        return (pack(arr[0], arr[1]), pack(arr[1], arr[2]),
                pack(arr[2], arr[0]))

    wAq, wBq, wXq = conv_w(emb_q)
    wAk, wBk, wXk = conv_w(emb_k)
    repl_ = np.tile(np.eye(CG, dtype=np.float32), (K7, K7))

    wq = np.asarray(wq, np.float32)
    wk = np.asarray(wk, np.float32)
    wv = np.asarray(wv, np.float32)

    in_maps = []
    for core in range(8):
        b, p = divmod(core, 2)
        wqT_ = np.zeros((256, 256), np.float32)
        wkT_ = np.zeros((256, 256), np.float32)
        for j in range(4):
            head = 2 * p + j - 1
            if 0 <= head < NH:
                wqT_[:, j * 64:(j + 1) * 64] = wq[head * 64:(head + 1) * 64, :].T
                wkT_[:, j * 64:(j + 1) * 64] = wk[head * 64:(head + 1) * 64, :].T
        wvT_ = wv[p * 128:(p + 1) * 128, :].T

        blob = np.zeros((128, CBLOB), np.float32)
        blob[:, OFF_WQ:OFF_WQ + 512] = \
            wqT_.reshape(2, 128, 256).transpose(1, 0, 2).reshape(128, 512)
        blob[:, OFF_WK:OFF_WK + 512] = \
            wkT_.reshape(2, 128, 256).transpose(1, 0, 2).reshape(128, 512)
        blob[:, OFF_WV:OFF_WV + 256] = \
            wvT_.reshape(2, 128, 128).transpose(1, 0, 2).reshape(128, 256)
        blob[:, OFF_XKV:OFF_XKV + 2048] = \
            xkv[b].reshape(2, 128, SK).transpose(1, 0, 2).reshape(128, 2048)
        cws = np.stack([wAq, wBq, wXq, wAk, wBk, wXk], axis=1)  # [128,6,7,128]
        blob[:, OFF_CW:OFF_CW + 6 * 896] = cws.reshape(128, 6 * 896)
        blob[0:MC, OFF_REPL:OFF_REPL + MC] = repl_

        in_maps.append(dict(
            blob=np.ascontiguousarray(blob.astype(ml_dtypes.bfloat16)),
            xq=np.ascontiguousarray(
                xq[b].reshape(256, SQ).astype(ml_dtypes.bfloat16)),
        ))
    return in_maps


def _run(inputs, **kw):
    nc = _get_program()
    in_maps = _host_inputs(**inputs)
    res = run_bass_kernel_spmd(nc, in_maps, core_ids=list(range(8)), **kw)
    outp = np.empty((B, 256, HQ, HQ), np.float32)
    for core in range(8):
        b, p = divmod(core, 2)
        outp[b, p * 128:(p + 1) * 128] = \
            res.results[core]["out"].reshape(128, HQ, HQ)
    return outp, res


def kernel(xq, xkv, wq, wk, wv, emb_q, emb_k):
    outp, _ = _run(dict(xq=xq, xkv=xkv, wq=wq, wk=wk, wv=wv,
                        emb_q=emb_q, emb_k=emb_k))
    return outp
